# revision 8
# baseline (speedup 1.0000x reference)
"""Trainium2 Bass kernel for ConstraintViolationLoss (GNN message passing).

Strategy (8 NeuronCores, SPMD):
  - Host does index-only layout prep: sort edges by constraint, classify each
    constraint into a degree tier (stride 16/24/32/40/48/96...), assign every
    constraint to one of 1024 (core, partition) bins, and pad each
    constraint's edge list to its tier stride so the per-constraint
    segment-sum becomes a fixed-stride reduction.
  - Launch 1 (8 cores, sharded over the small-int vars): softmax
    expected-value head: expected = softmax(logits) @ [0..C) + offsets.
  - Host assembles the per-edge-slot gathered value stream xg (pure indexed
    copies of input values / launch-1 output; no arithmetic).
  - Launch 2 (8 cores, constraints sharded into bins): w = xg * feat,
    per-segment strided reduce -> Ax, violation = relu(Ax - bias), then
    per-partition sum / max / count partials.
  - Host combines 1024 partial rows into the 4 scalar outputs.
"""

import sys

sys.path.insert(0, "/opt/trn_rl_repo")

import numpy as np

import concourse.bass as bass
import concourse.mybir as mybir
from concourse.bass_utils import run_bass_kernel_spmd

P = 128
NCORES = 8
NBINS = P * NCORES
LAMBDA_MEAN, LAMBDA_MAX = 1.0, 0.1
BIAS_COL = 1
LP_SOL_COL = 8
TIER_LS = [16, 24, 32, 40, 48]   # degree-class strides; overflow tier appended
CHUNK_ELEMS = 3072               # target elems/partition per pipelined chunk
BIG_BIAS = 1.0e30
F32 = mybir.dt.float32

# shapes of the most recent build, for test harness introspection
LAST_ROWS_PP = None
LAST_P2_ARGS = None
LA = 48  # legacy alias used by older validation scripts


def _build_phase1(rows_pp: int, nchunks: int = 4, ccls: int = 16):
    """expected = softmax(logits) @ cls + offsets; rows_pp rows/partition."""
    global LAST_ROWS_PP
    LAST_ROWS_PP = rows_pp
    assert rows_pp % nchunks == 0
    rc = rows_pp // nchunks
    fc = rc * ccls
    nf = rows_pp * ccls
    nc = bass.Bass()
    lg = nc.declare_dram_parameter("logits", [P, nf], F32, isOutput=False)
    cp = nc.declare_dram_parameter("clspat", [P, fc], F32, isOutput=False)
    of = nc.declare_dram_parameter("offs", [P, rows_pp], F32, isOutput=False)
    ex = nc.declare_dram_parameter("expected", [P, rows_pp], F32, isOutput=True)

    with (
        nc.sbuf_tensor([P, 2, fc], F32) as tl,
        nc.sbuf_tensor([P, 2, rc], F32) as tof,
        nc.sbuf_tensor([P, fc], F32) as tcp,
        nc.sbuf_tensor([P, fc], F32) as te,
        nc.sbuf_tensor([P, rc], F32) as tden,
        nc.sbuf_tensor([P, rc], F32) as tnum,
        nc.sbuf_tensor([P, rows_pp], F32) as tout,
        nc.Block() as block,
        nc.semaphore("pl0") as pl0,
        nc.semaphore("pl1") as pl1,
        nc.semaphore("csem") as csem,
        nc.semaphore("ssem") as ssem,
        nc.semaphore("vsem") as vsem,
        nc.semaphore("osem") as osem,
    ):
        pl = [pl0, pl1]

        @block.sync
        def _(sync):
            sync.dma_start(out=tcp[:], in_=cp[:]).then_inc(csem, 16)
            for c in range(nchunks):
                if c >= 2:
                    sync.wait_ge(vsem, c - 1)
                b = c % 2
                sync.dma_start(
                    out=tl[:, b, :], in_=lg[:, c * fc : (c + 1) * fc]
                ).then_inc(pl[b], 16)
                sync.dma_start(
                    out=tof[:, b, :], in_=of[:, c * rc : (c + 1) * rc]
                ).then_inc(pl[b], 16)
            sync.wait_ge(vsem, nchunks)
            sync.dma_start(out=ex[:], in_=tout[:]).then_inc(osem, 16)
            sync.wait_ge(osem, 16)

        @block.scalar
        def _(scalar):
            for c in range(nchunks):
                b = c % 2
                scalar.wait_ge(pl[b], 32 * (c // 2 + 1))
                nc.scalar.activation(
                    out=tl[:, b, :], in_=tl[:, b, :],
                    func=mybir.ActivationFunctionType.Exp,
                ).then_inc(ssem, 1)

        @block.vector
        def _(vector):
            vector.wait_ge(csem, 16)
            for c in range(nchunks):
                b = c % 2
                vector.wait_ge(ssem, c + 1)
                nc.vector.drain()
                g = tl[:, b, :].rearrange("p (r c) -> p r c", c=ccls)
                nc.vector.tensor_reduce(
                    out=tden[:], in_=g,
                    axis=mybir.AxisListType.X, op=mybir.AluOpType.add,
                )
                nc.vector.tensor_tensor(
                    out=te[:], in0=tl[:, b, :], in1=tcp[:],
                    op=mybir.AluOpType.mult,
                )
                nc.vector.drain()
                nc.vector.tensor_reduce(
                    out=tnum[:],
                    in_=te[:].rearrange("p (r c) -> p r c", c=ccls),
                    axis=mybir.AxisListType.X, op=mybir.AluOpType.add,
                )
                nc.vector.reciprocal(out=tden[:], in_=tden[:])
                nc.vector.drain()
                nc.vector.tensor_tensor(
                    out=tnum[:], in0=tnum[:], in1=tden[:],
                    op=mybir.AluOpType.mult,
                )
                nc.vector.drain()
                nc.vector.tensor_tensor(
                    out=tout[:, c * rc : (c + 1) * rc],
                    in0=tnum[:], in1=tof[:, b, :], op=mybir.AluOpType.add,
                )
                nc.vector.drain().then_inc(vsem, 1)

    return nc


def _build_phase2(tiers):
    """Per-core segment reduce + loss partials.

    tiers: list of (sa, L, ca) — segments/partition, stride, chunk segments.
    """
    global LAST_P2_ARGS
    LAST_P2_ARGS = (tiers,)
    nc = bass.Bass()
    xg, ft, bs = [], [], []
    for r, (sa, L, ca) in enumerate(tiers):
        xg.append(nc.declare_dram_parameter(f"xg{r}", [P, sa * L], F32, False))
        ft.append(nc.declare_dram_parameter(f"ft{r}", [P, sa * L], F32, False))
        bs.append(nc.declare_dram_parameter(f"bs{r}", [P, sa], F32, False))
    out_p = nc.declare_dram_parameter("partials", [P, 4], F32, isOutput=True)

    fmax = max(ca * L for sa, L, ca in tiers)
    cmax = max(ca for sa, L, ca in tiers)
    chunks = []  # (tier, chunk_idx)
    for r, (sa, L, ca) in enumerate(tiers):
        for i in range(sa // ca):
            chunks.append((r, i))

    with (
        nc.sbuf_tensor([P, 2, fmax], F32) as tx,
        nc.sbuf_tensor([P, 2, fmax], F32) as tf,
        nc.sbuf_tensor([P, 2, cmax], F32) as tb,
        nc.sbuf_tensor([P, cmax], F32) as tax,
        nc.sbuf_tensor([P, cmax], F32) as tviol,
        nc.sbuf_tensor([P, cmax], F32) as tgt,
        nc.sbuf_tensor([P, 1], F32) as ts,
        nc.sbuf_tensor([P, 1], F32) as asum,
        nc.sbuf_tensor([P, 1], F32) as amax,
        nc.sbuf_tensor([P, 1], F32) as acnt,
        nc.sbuf_tensor([P, 4], F32) as tout,
        nc.Block() as block,
        nc.semaphore("pa0") as pa0,
        nc.semaphore("pa1") as pa1,
        nc.semaphore("osem") as osem,
        nc.semaphore("vsem") as vsem,
    ):
        pa = [pa0, pa1]

        @block.sync
        def _(sync):
            for g, (r, i) in enumerate(chunks):
                sa, L, ca = tiers[r]
                fc = ca * L
                if g >= 2:
                    sync.wait_ge(vsem, g - 1)
                b = g % 2
                sync.dma_start(
                    out=tx[:, b, :fc], in_=xg[r][:, i * fc : (i + 1) * fc]
                ).then_inc(pa[b], 16)
                sync.dma_start(
                    out=tf[:, b, :fc], in_=ft[r][:, i * fc : (i + 1) * fc]
                ).then_inc(pa[b], 16)
                sync.dma_start(
                    out=tb[:, b, :ca], in_=bs[r][:, i * ca : (i + 1) * ca]
                ).then_inc(pa[b], 16)
            sync.wait_ge(vsem, len(chunks) + 1)
            sync.dma_start(out=out_p[:], in_=tout[:]).then_inc(osem, 16)
            sync.wait_ge(osem, 16)

        @block.vector
        def _(vector):
            nc.vector.memset(asum[:], 0.0)
            nc.vector.memset(amax[:], 0.0)
            nc.vector.memset(acnt[:], 0.0)

            def seg_chunk(xa, fa_, ba, nseg, ls):
                """Accumulate violation stats for nseg segments of stride ls."""
                nc.vector.drain()
                nc.vector.tensor_tensor(
                    out=xa, in0=xa, in1=fa_, op=mybir.AluOpType.mult
                )
                nc.vector.drain()
                nc.vector.tensor_reduce(
                    out=tax[:, :nseg],
                    in_=xa.rearrange("p (s l) -> p s l", l=ls),
                    axis=mybir.AxisListType.X, op=mybir.AluOpType.add,
                )
                nc.vector.drain()
                nc.vector.tensor_tensor(
                    out=tviol[:, :nseg], in0=tax[:, :nseg], in1=ba,
                    op=mybir.AluOpType.subtract,
                )
                nc.vector.drain()
                nc.vector.tensor_scalar_max(
                    out=tviol[:, :nseg], in0=tviol[:, :nseg], scalar1=0.0
                )
                nc.vector.drain()
                nc.vector.tensor_reduce(
                    out=ts[:], in_=tviol[:, :nseg],
                    axis=mybir.AxisListType.X, op=mybir.AluOpType.add,
                )
                nc.vector.drain()
                nc.vector.tensor_tensor(
                    out=asum[:], in0=asum[:], in1=ts[:], op=mybir.AluOpType.add
                )
                nc.vector.drain()
                nc.vector.tensor_reduce(
                    out=ts[:], in_=tviol[:, :nseg],
                    axis=mybir.AxisListType.X, op=mybir.AluOpType.max,
                )
                nc.vector.drain()
                nc.vector.tensor_tensor(
                    out=amax[:], in0=amax[:], in1=ts[:], op=mybir.AluOpType.max
                )
                nc.vector.tensor_scalar(
                    out=tgt[:, :nseg], in0=tviol[:, :nseg],
                    scalar1=1e-6, scalar2=None, op0=mybir.AluOpType.is_gt,
                )
                nc.vector.drain()
                nc.vector.tensor_reduce(
                    out=ts[:], in_=tgt[:, :nseg],
                    axis=mybir.AxisListType.X, op=mybir.AluOpType.add,
                )
                nc.vector.drain()
                nc.vector.tensor_tensor(
                    out=acnt[:], in0=acnt[:], in1=ts[:], op=mybir.AluOpType.add
                )

            for g, (r, i) in enumerate(chunks):
                sa, L, ca = tiers[r]
                fc = ca * L
                b = g % 2
                vector.wait_ge(pa[b], 48 * (g // 2 + 1))
                seg_chunk(tx[:, b, :fc], tf[:, b, :fc], tb[:, b, :ca], ca, L)
                nc.vector.drain().then_inc(vsem, 1)
            nc.vector.tensor_copy(out=tout[:, 0:1], in_=asum[:])
            nc.vector.tensor_copy(out=tout[:, 1:2], in_=amax[:])
            nc.vector.tensor_copy(out=tout[:, 2:3], in_=acnt[:])
            nc.vector.tensor_copy(out=tout[:, 3:4], in_=acnt[:])
            nc.vector.drain().then_inc(vsem, 1)

    return nc


def _round_up(x: int, m: int) -> int:
    return (x + m - 1) // m * m


def kernel(**inputs) -> tuple:
    prob_bin = np.asarray(inputs["prob_bin"], dtype=np.float32)
    logits = np.asarray(inputs["logits_int_small"], dtype=np.float32)
    offsets = np.asarray(inputs["int_small_offsets"], dtype=np.float32)
    pred_l = np.asarray(inputs["pred_int_large"], dtype=np.float32)
    feat = np.asarray(inputs["edge_features"], dtype=np.float32).reshape(-1)
    cfeat = np.asarray(inputs["constraint_features"], dtype=np.float32)
    vfeat = np.asarray(inputs["variable_features"], dtype=np.float32)
    idx_bin = np.asarray(inputs["idx_bin"], dtype=np.int64)
    idx_s = np.asarray(inputs["idx_int_small"], dtype=np.int64)
    idx_l = np.asarray(inputs["idx_int_large"], dtype=np.int64)
    var_types = np.asarray(inputs["var_types"], dtype=np.int64)
    ei = np.asarray(inputs["edge_indices"], dtype=np.int64)
    n_vars = int(inputs["n_vars"])

    n_con = cfeat.shape[0]
    ns, ccls = logits.shape
    bias = np.ascontiguousarray(cfeat[:, BIAS_COL])
    lp_vals = np.ascontiguousarray(vfeat[:, LP_SOL_COL])
    con = ei[0]
    var = ei[1]
    ne = con.shape[0]

    # ---------------- host index prep (layout only) ----------------
    deg = np.bincount(con, minlength=n_con)
    order = np.argsort(con, kind="stable")
    run_start = np.zeros(n_con + 1, dtype=np.int64)
    np.cumsum(deg, out=run_start[1:])
    off_in_run = np.arange(ne, dtype=np.int64) - run_start[con[order]]
    con_sorted = con[order]
    var_sorted = var[order].astype(np.int32)
    feat_sorted = feat[order]

    max_deg = int(deg.max()) if ne else 0
    strides = list(TIER_LS)
    if max_deg > strides[-1]:
        strides.append(max(96, _round_up(max_deg, 16)))
    # tier id per constraint: first stride >= deg
    tier_of = np.searchsorted(np.asarray(strides), deg, side="left")

    tiers = []        # (sa, L, ca) per tier with any segments
    tier_remap = {}   # original stride index -> dense tier index
    bin_of = np.zeros(n_con, dtype=np.int64)
    rank_of = np.zeros(n_con, dtype=np.int64)
    for si, L in enumerate(strides):
        cons = np.nonzero(tier_of == si)[0]
        if cons.size == 0:
            continue
        rank_order = cons[np.argsort(-deg[cons], kind="stable")]
        ar = np.arange(rank_order.size, dtype=np.int64)
        bin_of[rank_order] = ar % NBINS
        rank_of[rank_order] = ar // NBINS
        sa_need = max(int((rank_order.size + NBINS - 1) // NBINS), 1)
        n_chunks = max(1, -(-sa_need * L // CHUNK_ELEMS))
        ca = -(-sa_need // n_chunks)
        sa = ca * n_chunks
        tier_remap[si] = len(tiers)
        tiers.append((sa, L, ca))

    # per-edge destination slots, per tier
    e_tier = tier_of[con_sorted]
    xgv, ftv, bsv = [], [], []
    for si, r in sorted(tier_remap.items()):
        sa, L, ca = tiers[r]
        sel = e_tier == si
        cs = con_sorted[sel]
        idx = (bin_of[cs] * sa + rank_of[cs]) * L + off_in_run[sel]
        ftr = np.zeros(NBINS * sa * L, dtype=np.float32)
        varr = np.zeros(NBINS * sa * L, dtype=np.int32)
        ftr[idx] = feat_sorted[sel]
        varr[idx] = var_sorted[sel]
        cons = np.nonzero(tier_of == si)[0]
        bsr = np.full(NBINS * sa, BIG_BIAS, dtype=np.float32)
        bsr[bin_of[cons] * sa + rank_of[cons]] = bias[cons]
        ftv.append(ftr)
        xgv.append(varr)
        bsv.append(bsr)

    # ---------------- launch 1: expected values ----------------
    nch1 = 4
    rows_pp = _round_up((ns + NCORES * P - 1) // (NCORES * P), nch1)
    ns_pad = NCORES * P * rows_pp
    lg_pad = np.zeros((ns_pad, ccls), dtype=np.float32)
    lg_pad[:ns] = logits
    of_pad = np.zeros(ns_pad, dtype=np.float32)
    of_pad[:ns] = offsets
    rc = rows_pp // nch1
    clspat = np.tile(np.arange(ccls, dtype=np.float32), rc)[None].repeat(P, 0)

    nc1 = _build_phase1(rows_pp, nch1, ccls)
    lg_r = lg_pad.reshape(NCORES, P, rows_pp * ccls)
    of_r = of_pad.reshape(NCORES, P, rows_pp)
    in1 = [
        {"logits": lg_r[c], "clspat": clspat, "offs": of_r[c]} for c in range(NCORES)
    ]
    res1 = run_bass_kernel_spmd(nc1, in1, list(range(NCORES)))
    expected = np.concatenate(
        [res1.results[c]["expected"].reshape(-1) for c in range(NCORES)]
    )[:ns]

    # ---------------- host: assemble x and gather streams ----------------
    xfull = np.zeros(n_vars, dtype=np.float32)
    xfull[idx_bin] = prob_bin[:, 0]
    xfull[idx_s] = expected
    xfull[idx_l] = pred_l[:, 0]
    xfull = np.where(var_types == 0, lp_vals, xfull)

    # ---------------- launch 2: segment reduce + loss partials ----------------
    nc2 = _build_phase2(tiers)
    in2 = []
    for c in range(NCORES):
        m = {}
        for r, (sa, L, ca) in enumerate(tiers):
            m[f"xg{r}"] = xfull[xgv[r].reshape(NCORES, P, sa * L)[c]]
            m[f"ft{r}"] = ftv[r].reshape(NCORES, P, sa * L)[c]
            m[f"bs{r}"] = bsv[r].reshape(NCORES, P, sa)[c]
        in2.append(m)
    res2 = run_bass_kernel_spmd(nc2, in2, list(range(NCORES)))

    parts = np.stack([res2.results[c]["partials"] for c in range(NCORES)])
    vsum = np.float32(parts[:, :, 0].astype(np.float64).sum())
    vmax = np.float32(parts[:, :, 1].max())
    vcnt = np.int32(round(float(parts[:, :, 2].sum())))
    mean_viol = np.float32(vsum / np.float32(n_con))
    penalty = np.float32(
        np.float32(LAMBDA_MEAN) * mean_viol + np.float32(LAMBDA_MAX) * vmax
    )
    return penalty, mean_viol, vmax, vcnt


# revision 9
# speedup vs baseline: 1.0073x; 1.0073x over previous
"""Trainium2 Bass kernel for ConstraintViolationLoss (GNN message passing).

Strategy (8 NeuronCores, SPMD):
  - Host does index-only layout prep: sort edges by constraint, classify each
    constraint into a degree tier (stride 16/24/32/40/48/96...), assign every
    constraint to one of 1024 (core, partition) bins, and pad each
    constraint's edge list to its tier stride so the per-constraint
    segment-sum becomes a fixed-stride reduction.
  - Launch 1 (8 cores, sharded over the small-int vars): softmax
    expected-value head: expected = softmax(logits) @ [0..C) + offsets.
  - Host assembles the per-edge-slot gathered value stream xg (pure indexed
    copies of input values / launch-1 output; no arithmetic).
  - Launch 2 (8 cores, constraints sharded into bins): w = xg * feat,
    per-segment strided reduce -> Ax, violation = relu(Ax - bias), then
    per-partition sum / max / count partials.
  - Host combines 1024 partial rows into the 4 scalar outputs.
"""

import sys

sys.path.insert(0, "/opt/trn_rl_repo")

import numpy as np

import concourse.bass as bass
import concourse.mybir as mybir
from concourse.bass_utils import run_bass_kernel_spmd

P = 128
NCORES = 8
NBINS = P * NCORES
LAMBDA_MEAN, LAMBDA_MAX = 1.0, 0.1
BIAS_COL = 1
LP_SOL_COL = 8
TIER_LS = [16, 24, 32, 40, 48]   # degree-class strides; overflow tier appended
CHUNK_ELEMS = 6144               # target elems/partition per pipelined chunk
BIG_BIAS = 1.0e30
F32 = mybir.dt.float32

# shapes of the most recent build, for test harness introspection
LAST_ROWS_PP = None
LAST_P2_ARGS = None
LA = 48  # legacy alias used by older validation scripts


def _build_phase1(rows_pp: int, nchunks: int = 4, ccls: int = 16):
    """expected = softmax(logits) @ cls + offsets; rows_pp rows/partition."""
    global LAST_ROWS_PP
    LAST_ROWS_PP = rows_pp
    assert rows_pp % nchunks == 0
    rc = rows_pp // nchunks
    fc = rc * ccls
    nf = rows_pp * ccls
    nc = bass.Bass()
    lg = nc.declare_dram_parameter("logits", [P, nf], F32, isOutput=False)
    cp = nc.declare_dram_parameter("clspat", [P, fc], F32, isOutput=False)
    of = nc.declare_dram_parameter("offs", [P, rows_pp], F32, isOutput=False)
    ex = nc.declare_dram_parameter("expected", [P, rows_pp], F32, isOutput=True)

    with (
        nc.sbuf_tensor([P, 2, fc], F32) as tl,
        nc.sbuf_tensor([P, 2, rc], F32) as tof,
        nc.sbuf_tensor([P, fc], F32) as tcp,
        nc.sbuf_tensor([P, fc], F32) as te,
        nc.sbuf_tensor([P, rc], F32) as tden,
        nc.sbuf_tensor([P, rc], F32) as tnum,
        nc.sbuf_tensor([P, rows_pp], F32) as tout,
        nc.Block() as block,
        nc.semaphore("pl0") as pl0,
        nc.semaphore("pl1") as pl1,
        nc.semaphore("csem") as csem,
        nc.semaphore("ssem") as ssem,
        nc.semaphore("vsem") as vsem,
        nc.semaphore("osem") as osem,
    ):
        pl = [pl0, pl1]

        @block.sync
        def _(sync):
            sync.dma_start(out=tcp[:], in_=cp[:]).then_inc(csem, 16)
            for c in range(nchunks):
                if c >= 2:
                    sync.wait_ge(vsem, c - 1)
                b = c % 2
                sync.dma_start(
                    out=tl[:, b, :], in_=lg[:, c * fc : (c + 1) * fc]
                ).then_inc(pl[b], 16)
                sync.dma_start(
                    out=tof[:, b, :], in_=of[:, c * rc : (c + 1) * rc]
                ).then_inc(pl[b], 16)
            sync.wait_ge(vsem, nchunks)
            sync.dma_start(out=ex[:], in_=tout[:]).then_inc(osem, 16)
            sync.wait_ge(osem, 16)

        @block.scalar
        def _(scalar):
            for c in range(nchunks):
                b = c % 2
                scalar.wait_ge(pl[b], 32 * (c // 2 + 1))
                nc.scalar.activation(
                    out=tl[:, b, :], in_=tl[:, b, :],
                    func=mybir.ActivationFunctionType.Exp,
                ).then_inc(ssem, 1)

        @block.vector
        def _(vector):
            vector.wait_ge(csem, 16)
            for c in range(nchunks):
                b = c % 2
                vector.wait_ge(ssem, c + 1)
                nc.vector.drain()
                g = tl[:, b, :].rearrange("p (r c) -> p r c", c=ccls)
                nc.vector.tensor_reduce(
                    out=tden[:], in_=g,
                    axis=mybir.AxisListType.X, op=mybir.AluOpType.add,
                )
                nc.vector.tensor_tensor(
                    out=te[:], in0=tl[:, b, :], in1=tcp[:],
                    op=mybir.AluOpType.mult,
                )
                nc.vector.drain()
                nc.vector.tensor_reduce(
                    out=tnum[:],
                    in_=te[:].rearrange("p (r c) -> p r c", c=ccls),
                    axis=mybir.AxisListType.X, op=mybir.AluOpType.add,
                )
                nc.vector.reciprocal(out=tden[:], in_=tden[:])
                nc.vector.drain()
                nc.vector.tensor_tensor(
                    out=tnum[:], in0=tnum[:], in1=tden[:],
                    op=mybir.AluOpType.mult,
                )
                nc.vector.drain()
                nc.vector.tensor_tensor(
                    out=tout[:, c * rc : (c + 1) * rc],
                    in0=tnum[:], in1=tof[:, b, :], op=mybir.AluOpType.add,
                )
                nc.vector.drain().then_inc(vsem, 1)

    return nc


def _build_phase2(tiers):
    """Per-core segment reduce + loss partials.

    tiers: list of (sa, L, ca) — segments/partition, stride, chunk segments.
    """
    global LAST_P2_ARGS
    LAST_P2_ARGS = (tiers,)
    nc = bass.Bass()
    xg, ft, bs = [], [], []
    for r, (sa, L, ca) in enumerate(tiers):
        xg.append(nc.declare_dram_parameter(f"xg{r}", [P, sa * L], F32, False))
        ft.append(nc.declare_dram_parameter(f"ft{r}", [P, sa * L], F32, False))
        bs.append(nc.declare_dram_parameter(f"bs{r}", [P, sa], F32, False))
    out_p = nc.declare_dram_parameter("partials", [P, 4], F32, isOutput=True)

    fmax = max(ca * L for sa, L, ca in tiers)
    cmax = max(ca for sa, L, ca in tiers)
    chunks = []  # (tier, chunk_idx)
    for r, (sa, L, ca) in enumerate(tiers):
        for i in range(sa // ca):
            chunks.append((r, i))

    with (
        nc.sbuf_tensor([P, 2, fmax], F32) as tx,
        nc.sbuf_tensor([P, 2, fmax], F32) as tf,
        nc.sbuf_tensor([P, 2, cmax], F32) as tb,
        nc.sbuf_tensor([P, cmax], F32) as tax,
        nc.sbuf_tensor([P, cmax], F32) as tviol,
        nc.sbuf_tensor([P, cmax], F32) as tgt,
        nc.sbuf_tensor([P, 1], F32) as ts,
        nc.sbuf_tensor([P, 1], F32) as ts2,
        nc.sbuf_tensor([P, 1], F32) as ts3,
        nc.sbuf_tensor([P, 1], F32) as asum,
        nc.sbuf_tensor([P, 1], F32) as amax,
        nc.sbuf_tensor([P, 1], F32) as acnt,
        nc.sbuf_tensor([P, 4], F32) as tout,
        nc.Block() as block,
        nc.semaphore("pa0") as pa0,
        nc.semaphore("pa1") as pa1,
        nc.semaphore("osem") as osem,
        nc.semaphore("vsem") as vsem,
    ):
        pa = [pa0, pa1]

        @block.sync
        def _(sync):
            for g, (r, i) in enumerate(chunks):
                sa, L, ca = tiers[r]
                fc = ca * L
                if g >= 2:
                    sync.wait_ge(vsem, g - 1)
                b = g % 2
                sync.dma_start(
                    out=tx[:, b, :fc], in_=xg[r][:, i * fc : (i + 1) * fc]
                ).then_inc(pa[b], 16)
                sync.dma_start(
                    out=tf[:, b, :fc], in_=ft[r][:, i * fc : (i + 1) * fc]
                ).then_inc(pa[b], 16)
                sync.dma_start(
                    out=tb[:, b, :ca], in_=bs[r][:, i * ca : (i + 1) * ca]
                ).then_inc(pa[b], 16)
            sync.wait_ge(vsem, len(chunks) + 1)
            sync.dma_start(out=out_p[:], in_=tout[:]).then_inc(osem, 16)
            sync.wait_ge(osem, 16)

        @block.vector
        def _(vector):
            nc.vector.memset(asum[:], 0.0)
            nc.vector.memset(amax[:], 0.0)
            nc.vector.memset(acnt[:], 0.0)

            def seg_chunk(xa, fa_, ba, nseg, ls):
                """Accumulate violation stats for nseg segments of stride ls."""
                nc.vector.drain()
                nc.vector.tensor_tensor(
                    out=xa, in0=xa, in1=fa_, op=mybir.AluOpType.mult
                )
                nc.vector.drain()
                nc.vector.tensor_reduce(
                    out=tax[:, :nseg],
                    in_=xa.rearrange("p (s l) -> p s l", l=ls),
                    axis=mybir.AxisListType.X, op=mybir.AluOpType.add,
                )
                nc.vector.drain()
                nc.vector.tensor_tensor(
                    out=tviol[:, :nseg], in0=tax[:, :nseg], in1=ba,
                    op=mybir.AluOpType.subtract,
                )
                nc.vector.drain()
                nc.vector.tensor_scalar_max(
                    out=tviol[:, :nseg], in0=tviol[:, :nseg], scalar1=0.0
                )
                nc.vector.drain()
                # the three reads of tviol are independent of each other
                nc.vector.tensor_reduce(
                    out=ts[:], in_=tviol[:, :nseg],
                    axis=mybir.AxisListType.X, op=mybir.AluOpType.add,
                )
                nc.vector.tensor_reduce(
                    out=ts2[:], in_=tviol[:, :nseg],
                    axis=mybir.AxisListType.X, op=mybir.AluOpType.max,
                )
                nc.vector.tensor_scalar(
                    out=tgt[:, :nseg], in0=tviol[:, :nseg],
                    scalar1=1e-6, scalar2=None, op0=mybir.AluOpType.is_gt,
                )
                nc.vector.drain()
                nc.vector.tensor_tensor(
                    out=asum[:], in0=asum[:], in1=ts[:], op=mybir.AluOpType.add
                )
                nc.vector.tensor_tensor(
                    out=amax[:], in0=amax[:], in1=ts2[:], op=mybir.AluOpType.max
                )
                nc.vector.tensor_reduce(
                    out=ts3[:], in_=tgt[:, :nseg],
                    axis=mybir.AxisListType.X, op=mybir.AluOpType.add,
                )
                nc.vector.drain()
                nc.vector.tensor_tensor(
                    out=acnt[:], in0=acnt[:], in1=ts3[:], op=mybir.AluOpType.add
                )

            for g, (r, i) in enumerate(chunks):
                sa, L, ca = tiers[r]
                fc = ca * L
                b = g % 2
                vector.wait_ge(pa[b], 48 * (g // 2 + 1))
                seg_chunk(tx[:, b, :fc], tf[:, b, :fc], tb[:, b, :ca], ca, L)
                nc.vector.drain().then_inc(vsem, 1)
            nc.vector.tensor_copy(out=tout[:, 0:1], in_=asum[:])
            nc.vector.tensor_copy(out=tout[:, 1:2], in_=amax[:])
            nc.vector.tensor_copy(out=tout[:, 2:3], in_=acnt[:])
            nc.vector.tensor_copy(out=tout[:, 3:4], in_=acnt[:])
            nc.vector.drain().then_inc(vsem, 1)

    return nc


def _round_up(x: int, m: int) -> int:
    return (x + m - 1) // m * m


def kernel(**inputs) -> tuple:
    prob_bin = np.asarray(inputs["prob_bin"], dtype=np.float32)
    logits = np.asarray(inputs["logits_int_small"], dtype=np.float32)
    offsets = np.asarray(inputs["int_small_offsets"], dtype=np.float32)
    pred_l = np.asarray(inputs["pred_int_large"], dtype=np.float32)
    feat = np.asarray(inputs["edge_features"], dtype=np.float32).reshape(-1)
    cfeat = np.asarray(inputs["constraint_features"], dtype=np.float32)
    vfeat = np.asarray(inputs["variable_features"], dtype=np.float32)
    idx_bin = np.asarray(inputs["idx_bin"], dtype=np.int64)
    idx_s = np.asarray(inputs["idx_int_small"], dtype=np.int64)
    idx_l = np.asarray(inputs["idx_int_large"], dtype=np.int64)
    var_types = np.asarray(inputs["var_types"], dtype=np.int64)
    ei = np.asarray(inputs["edge_indices"], dtype=np.int64)
    n_vars = int(inputs["n_vars"])

    n_con = cfeat.shape[0]
    ns, ccls = logits.shape
    bias = np.ascontiguousarray(cfeat[:, BIAS_COL])
    lp_vals = np.ascontiguousarray(vfeat[:, LP_SOL_COL])
    con = ei[0]
    var = ei[1]
    ne = con.shape[0]

    # ---------------- host index prep (layout only) ----------------
    deg = np.bincount(con, minlength=n_con)
    order = np.argsort(con, kind="stable")
    run_start = np.zeros(n_con + 1, dtype=np.int64)
    np.cumsum(deg, out=run_start[1:])
    off_in_run = np.arange(ne, dtype=np.int64) - run_start[con[order]]
    con_sorted = con[order]
    var_sorted = var[order].astype(np.int32)
    feat_sorted = feat[order]

    max_deg = int(deg.max()) if ne else 0
    strides = list(TIER_LS)
    if max_deg > strides[-1]:
        strides.append(max(96, _round_up(max_deg, 16)))
    # tier id per constraint: first stride >= deg
    tier_of = np.searchsorted(np.asarray(strides), deg, side="left")

    tiers = []        # (sa, L, ca) per tier with any segments
    tier_remap = {}   # original stride index -> dense tier index
    bin_of = np.zeros(n_con, dtype=np.int64)
    rank_of = np.zeros(n_con, dtype=np.int64)
    for si, L in enumerate(strides):
        cons = np.nonzero(tier_of == si)[0]
        if cons.size == 0:
            continue
        rank_order = cons[np.argsort(-deg[cons], kind="stable")]
        ar = np.arange(rank_order.size, dtype=np.int64)
        bin_of[rank_order] = ar % NBINS
        rank_of[rank_order] = ar // NBINS
        sa_need = max(int((rank_order.size + NBINS - 1) // NBINS), 1)
        n_chunks = max(1, -(-sa_need * L // CHUNK_ELEMS))
        ca = -(-sa_need // n_chunks)
        sa = ca * n_chunks
        tier_remap[si] = len(tiers)
        tiers.append((sa, L, ca))

    # per-edge destination slots, per tier
    e_tier = tier_of[con_sorted]
    xgv, ftv, bsv = [], [], []
    for si, r in sorted(tier_remap.items()):
        sa, L, ca = tiers[r]
        sel = e_tier == si
        cs = con_sorted[sel]
        idx = (bin_of[cs] * sa + rank_of[cs]) * L + off_in_run[sel]
        ftr = np.zeros(NBINS * sa * L, dtype=np.float32)
        varr = np.zeros(NBINS * sa * L, dtype=np.int32)
        ftr[idx] = feat_sorted[sel]
        varr[idx] = var_sorted[sel]
        cons = np.nonzero(tier_of == si)[0]
        bsr = np.full(NBINS * sa, BIG_BIAS, dtype=np.float32)
        bsr[bin_of[cons] * sa + rank_of[cons]] = bias[cons]
        ftv.append(ftr)
        xgv.append(varr)
        bsv.append(bsr)

    # ---------------- launch 1: expected values ----------------
    nch1 = 4
    rows_pp = _round_up((ns + NCORES * P - 1) // (NCORES * P), nch1)
    ns_pad = NCORES * P * rows_pp
    lg_pad = np.zeros((ns_pad, ccls), dtype=np.float32)
    lg_pad[:ns] = logits
    of_pad = np.zeros(ns_pad, dtype=np.float32)
    of_pad[:ns] = offsets
    rc = rows_pp // nch1
    clspat = np.tile(np.arange(ccls, dtype=np.float32), rc)[None].repeat(P, 0)

    nc1 = _build_phase1(rows_pp, nch1, ccls)
    lg_r = lg_pad.reshape(NCORES, P, rows_pp * ccls)
    of_r = of_pad.reshape(NCORES, P, rows_pp)
    in1 = [
        {"logits": lg_r[c], "clspat": clspat, "offs": of_r[c]} for c in range(NCORES)
    ]
    res1 = run_bass_kernel_spmd(nc1, in1, list(range(NCORES)))
    expected = np.concatenate(
        [res1.results[c]["expected"].reshape(-1) for c in range(NCORES)]
    )[:ns]

    # ---------------- host: assemble x and gather streams ----------------
    xfull = np.zeros(n_vars, dtype=np.float32)
    xfull[idx_bin] = prob_bin[:, 0]
    xfull[idx_s] = expected
    xfull[idx_l] = pred_l[:, 0]
    xfull = np.where(var_types == 0, lp_vals, xfull)

    # ---------------- launch 2: segment reduce + loss partials ----------------
    nc2 = _build_phase2(tiers)
    in2 = []
    for c in range(NCORES):
        m = {}
        for r, (sa, L, ca) in enumerate(tiers):
            m[f"xg{r}"] = xfull[xgv[r].reshape(NCORES, P, sa * L)[c]]
            m[f"ft{r}"] = ftv[r].reshape(NCORES, P, sa * L)[c]
            m[f"bs{r}"] = bsv[r].reshape(NCORES, P, sa)[c]
        in2.append(m)
    res2 = run_bass_kernel_spmd(nc2, in2, list(range(NCORES)))

    parts = np.stack([res2.results[c]["partials"] for c in range(NCORES)])
    vsum = np.float32(parts[:, :, 0].astype(np.float64).sum())
    vmax = np.float32(parts[:, :, 1].max())
    vcnt = np.int32(round(float(parts[:, :, 2].sum())))
    mean_viol = np.float32(vsum / np.float32(n_con))
    penalty = np.float32(
        np.float32(LAMBDA_MEAN) * mean_viol + np.float32(LAMBDA_MAX) * vmax
    )
    return penalty, mean_viol, vmax, vcnt
